# revision 28
# baseline (speedup 1.0000x reference)
"""Block-diagonal matmul with softmax-normalized weights, SPMD on 8 NeuronCores.

Computes: out[b, n*128+o] = sum_m x[b, n*128+m] * softmax(c[n], axis=m)[m, o]
for n in 512 independent 128x128 blocks, b in 2048 batch rows.

Sharding: blocks are fully independent -> shard the n_blocks axis across the
8 cores (64 blocks per core). Each core sees x columns [i*8192, (i+1)*8192),
blocks c[i*64:(i+1)*64], and produces the matching output column slice.

The per-core c shard is repacked on the host to an m-major layout
[m=128, n*o=8192] so it lands in SBUF with one 4 MiB DMA (32 KiB per-partition
descriptors) already in the [m(partitions), o(free)] orientation the matmul
needs; the natural [n, m, o] layout would cost 8192 512-byte descriptors.

Per-core kernel (Tile framework), all fp32 (exact):
  Phase 1 (tiny): softmax weights for the core's 64 blocks, computed as
    w = exp(c - ln(colsum(exp(c)))). The column sums over m (the partition
    axis) come from a ones-matmul, which also broadcasts them to all 128
    partitions; Ln shares ScalarE's activation table with Exp (no table
    swaps) and reads the sums straight from PSUM, and VectorE only does the
    subtract — sidestepping both the slow VectorE reciprocal and the
    partition-broadcast problem. Max-subtraction is skipped: c ~ N(0,1), exp
    is safely in range, and the result matches fp32 softmax to ~1e-7.
  Phase 2 (bulk): for each (batch-tile, block): PE-transpose the x tile (the
    contraction dim m must sit on partitions for both matmul operands), in
    groups of 4 into one PSUM bank so VectorE evicts 4 tiles per copy; then
    fp32 matmul lhsT=xT, rhs=w_n writes the output tile in natural [b, o]
    layout, 8 blocks per 2-bank PSUM group evicted by one ScalarE copy; 2 MiB
    DMAs stream x in and the results out.
"""

import numpy as np
from contextlib import ExitStack

import concourse.bacc as bacc
import concourse.tile as tile
from concourse import mybir
from concourse.bass_utils import run_bass_kernel_spmd

F32 = mybir.dt.float32
P = 128
N_CORES = 8
N_BLOCKS_TOTAL = 512
BLOCKS_PER_CORE = N_BLOCKS_TOTAL // N_CORES  # 64
BATCH = 2048
XCOLS = BLOCKS_PER_CORE * P  # 8192
LAYER = N_BLOCKS_TOTAL * P   # 65536


def _body(tc, out, x, c, ident, batch, blocks):
    nc = tc.nc
    G1 = 4                      # blocks per softmax group (one PSUM bank)
    CHUNK = min(64, blocks)     # blocks per x chunk in phase 2 (4 MiB DMAs)
    OCT = min(8, CHUNK)         # blocks per output PSUM group (2 banks)
    QUAD = 4                    # blocks per transpose PSUM bank
    n_t = batch // P
    n_g = blocks // CHUNK

    with ExitStack() as ctx:
        # Phase-2 pools are allocated FIRST so their SBUF/PSUM zones do not
        # overlap the phase-1 scratch zones: with the stack allocator, a later
        # pool reusing a released zone inherits a dependency on every phase-1
        # instruction that touched it, which would stall the early x loads.
        const = ctx.enter_context(tc.tile_pool(name="const", bufs=1))
        ident_sb = const.tile([P, P], F32)
        nc.sync.dma_start(out=ident_sb[:], in_=ident)
        ones_sb = const.tile([P, P], F32)
        nc.vector.memset(ones_sb[:], 1.0)
        # Normalized weights, one tile per softmax group so phase-2 matmuls
        # only depend on their own group's writes.
        wpool = ctx.enter_context(tc.tile_pool(name="wpool", bufs=1))
        w_tiles = [wpool.tile([P, G1 * P], F32, name=f"w{g}", tag=f"w{g}")
                   for g in range(blocks // G1)]

        def w_slice(n):
            """AP for block n's weights [m, o]."""
            g, r = divmod(n, G1)
            return w_tiles[g][:, r * P:(r + 1) * P]

        xpool = ctx.enter_context(tc.tile_pool(name="xpool", bufs=2))
        xtpool = ctx.enter_context(tc.tile_pool(name="xtpool", bufs=4))
        opool = ctx.enter_context(tc.tile_pool(name="opool", bufs=2))
        psum_t = ctx.enter_context(tc.tile_pool(name="psum_t", bufs=3, space="PSUM"))
        psum_o = ctx.enter_context(tc.tile_pool(name="psum_o", bufs=2, space="PSUM"))

        # ---- Phase 1: softmax weights via w = exp(c - ln(colsum(exp(c)))) ----
        # Ln and Exp share an ACT table (no swaps), and Ln reads the column
        # sums straight from PSUM, so VectorE only does the subtracts. Each
        # 4-block group is an independent small-tile pipeline, so the first
        # weight groups are ready within a few microseconds and phase-2
        # matmuls can start almost immediately.
        with ExitStack() as p1:
            cpool = p1.enter_context(tc.tile_pool(name="cpool", bufs=1))
            epool = p1.enter_context(tc.tile_pool(name="epool", bufs=2))
            lnpool = p1.enter_context(tc.tile_pool(name="lnpool", bufs=2))
            subpool = p1.enter_context(tc.tile_pool(name="subpool", bufs=2))
            psum_s = p1.enter_context(tc.tile_pool(name="psum_s", bufs=1, space="PSUM"))
            CG = min(4, blocks // G1)   # softmax groups per c DMA (8 KiB rows)
            c_tiles = {}
            for g in range(blocks // G1):
                sl = slice(g * G1 * P, (g + 1) * G1 * P)
                if g % CG == 0:
                    ct_big = cpool.tile([P, CG * G1 * P], F32, name=f"c{g}",
                                        tag="cbig")
                    nc.sync.dma_start(
                        out=ct_big[:],
                        in_=c[:, g * G1 * P:(g + CG) * G1 * P],
                    )
                    c_tiles[g // CG] = ct_big
                ct = c_tiles[g // CG][:, (g % CG) * G1 * P:(g % CG + 1) * G1 * P]
                et = epool.tile([P, G1 * P], F32)
                nc.scalar.activation(et[:], ct,
                                     mybir.ActivationFunctionType.Exp)
                ps = psum_s.tile([P, G1 * P], F32)
                nc.tensor.matmul(ps[:], ones_sb[:], et[:], start=True, stop=True)
                lt = lnpool.tile([P, G1 * P], F32)
                nc.scalar.activation(lt[:], ps[:],
                                     mybir.ActivationFunctionType.Ln)
                st = subpool.tile([P, G1 * P], F32)
                nc.vector.tensor_tensor(st[:], ct[:], lt[:],
                                        op=mybir.AluOpType.subtract)
                nc.scalar.activation(w_tiles[g][:], st[:],
                                     mybir.ActivationFunctionType.Exp)

        # ---- Phase 2: block matmuls ----
        for t in range(n_t):
            for g in range(n_g):
                xt = xpool.tile([P, CHUNK * P], F32)
                nc.sync.dma_start(
                    out=xt[:],
                    in_=x[t * P:(t + 1) * P, g * CHUNK * P:(g + 1) * CHUNK * P],
                )
                ot = opool.tile([P, CHUNK * P], F32)
                for h in range(CHUNK // OCT):
                    pso = psum_o.tile([P, OCT * P], F32)
                    for q in range(OCT // QUAD):
                        pst = psum_t.tile([P, QUAD * P], F32)
                        for j in range(QUAD):
                            nb = h * OCT + q * QUAD + j
                            nc.tensor.transpose(
                                pst[:, j * P:(j + 1) * P],
                                xt[:, nb * P:(nb + 1) * P],
                                ident_sb[:],
                            )
                        xts = xtpool.tile([P, QUAD * P], F32)
                        nc.vector.tensor_copy(xts[:], pst[:])
                        for j in range(QUAD):
                            nb = h * OCT + q * QUAD + j
                            n = g * CHUNK + nb
                            nc.tensor.matmul(
                                pso[:, (q * QUAD + j) * P:(q * QUAD + j + 1) * P],
                                xts[:, j * P:(j + 1) * P],
                                w_slice(n),
                                start=True,
                                stop=True,
                            )
                    # ScalarE also runs phase-1's exp/ln chain at the start;
                    # route the first batch-tiles' evictions to VectorE (one
                    # PSUM bank per copy) so the drain doesn't stall there.
                    if t < 2:
                        for b2 in range(OCT * P // 512):
                            bsl = slice(h * OCT * P + b2 * 512,
                                        h * OCT * P + (b2 + 1) * 512)
                            nc.vector.tensor_copy(
                                ot[:, bsl], pso[:, b2 * 512:(b2 + 1) * 512])
                    else:
                        nc.scalar.copy(
                            ot[:, h * OCT * P:(h + 1) * OCT * P], pso[:])
                nc.sync.dma_start(
                    out=out[t * P:(t + 1) * P, g * CHUNK * P:(g + 1) * CHUNK * P],
                    in_=ot[:],
                )


def build_program(batch=BATCH, blocks=BLOCKS_PER_CORE):
    nc = bacc.Bacc("TRN2", target_bir_lowering=False, debug=False)
    xcols = blocks * P
    x = nc.dram_tensor("x", [batch, xcols], F32, kind="ExternalInput").ap()
    # c arrives host-repacked as [m, n*o] (m-major), see _make_in_maps.
    c = nc.dram_tensor("c", [P, blocks * P], F32, kind="ExternalInput").ap()
    ident = nc.dram_tensor("ident", [P, P], F32, kind="ExternalInput").ap()
    out = nc.dram_tensor("out", [batch, xcols], F32, kind="ExternalOutput").ap()
    with tile.TileContext(nc) as tc:
        _body(tc, out, x, c, ident, batch, blocks)
    nc.compile()
    return nc


_NC_CACHE = {}


def _get_nc():
    if "nc" not in _NC_CACHE:
        _NC_CACHE["nc"] = build_program()
    return _NC_CACHE["nc"]


def repack_c(c_shard):
    """[n, m, o] -> m-major [m, n*o] so the kernel's c DMA has 32 KiB rows."""
    n = c_shard.shape[0]
    return np.ascontiguousarray(
        c_shard.transpose(1, 0, 2).reshape(P, n * P)
    )


def _make_in_maps(x, c):
    ident = np.eye(P, dtype=np.float32)
    xr = x.reshape(BATCH, N_CORES, XCOLS)
    in_maps = []
    for i in range(N_CORES):
        in_maps.append(
            {
                "x": np.ascontiguousarray(xr[:, i, :]),
                "c": repack_c(c[i * BLOCKS_PER_CORE:(i + 1) * BLOCKS_PER_CORE]),
                "ident": ident,
            }
        )
    return in_maps


def run_on_hw(x, c, trace=False):
    """Run the SPMD kernel on the 8 cores; returns (out, BassKernelResults)."""
    x = np.asarray(x, dtype=np.float32)
    c = np.asarray(c, dtype=np.float32)
    assert x.shape == (BATCH, LAYER), x.shape
    assert c.shape == (N_BLOCKS_TOTAL, P, P), c.shape
    nc = _get_nc()
    in_maps = _make_in_maps(x, c)
    res = None
    for attempt in range(3):
        try:
            res = run_bass_kernel_spmd(
                nc, in_maps, core_ids=list(range(N_CORES)), trace=trace
            )
            break
        except Exception:
            # Transient runtime failures (e.g. a device flake) are rare but
            # fatal to a single attempt; retry with a fresh dispatch.
            if attempt == 2:
                raise
    assert res is not None
    out = np.empty((BATCH, LAYER), dtype=np.float32)
    orv = out.reshape(BATCH, N_CORES, XCOLS)
    for i in range(N_CORES):
        orv[:, i, :] = res.results[i]["out"]
    return out, res


def kernel(x, c):
    out, _ = run_on_hw(x, c, trace=False)
    return out
